# revision 23
# baseline (speedup 1.0000x reference)
"""Distributed Trainium2 kernel for nn_DecoderAttentionRotary.

Strategy (8 NeuronCores, tensor-parallel over heads, fp16 matmuls + fp8 et):
  - host: transpose x -> xT [D, B*L] fp16; per-core Wqkv column slice
    reordered to [q0,k0,q1,k1,v0|v1] fp16; cos/sin fp16; causal masks fp16.
  - device, per core (2 heads):
      phase 1: qkT = (Wqk^T @ xT) + b (fp16, double-wide [128,1024] psums),
               v = x @ Wv in [l, hd] layout; RoPE fused (DVE fp16 2x mode).
      phase 2: causal attention in scores^T layout; one exp per k-tile PAIR
               (ACT -> fp8e4 et, bias=-2 keeps fp8 range, cancels in norm);
               AV per tile (fp16 v x fp8 et), rowsums via one fp8-DoubleRow
               ones-matmul per pair; normalize = approx-reciprocal + gpsimd
               partition_broadcast; scores of pair i+1 emitted before AV i.
      A2A b0 after batch 0 (overlaps b1 attention); b1's A2A split per head
      (h0 half overlaps h1 attention).  phase-3 b0 units interleaved into
      the b1 attention emission (PE fills ACT-bound gaps).
      phase 3: y = o^T @ Wd + bd per (i, n4) unit; even-kt matmuls first so
      b1 units start after the h0 half-A2A; Wd streamed through 2 slots.
  - host: scatter the per-core 256-row halves into the full output.
"""
import sys

for _p in ("/opt/pypackages", "/opt/trn_rl_repo"):
    if _p not in sys.path:
        sys.path.insert(0, _p)

import numpy as np

B, L, D, H = 2, 2048, 2048, 16
HD, R = 128, 32
SCALE = float(HD) ** -0.5
EXPB = -2.0               # exp bias: keeps fp8e4 et in range; cancels in norm
W = 8
HPC = H // W              # heads per core
M = B * L                 # flattened rows
CORES = list(range(W))

_NC = None


def _build_nc():
    import concourse.mybir as mybir
    import concourse.tile as tile
    from concourse import bacc

    f32 = mybir.dt.float32
    f16 = mybir.dt.float16
    f8 = mybir.dt.float8e4
    DR = mybir.MatmulPerfMode.DoubleRow
    AFT = mybir.ActivationFunctionType
    OP = mybir.AluOpType

    nc = bacc.Bacc(None, target_bir_lowering=False, num_devices=W)
    xT = nc.declare_dram_parameter("xT", [D, M], f16, isOutput=False)
    wqkv = nc.declare_dram_parameter("wqkv", [D, 6 * HD], f16, isOutput=False)
    bqk = nc.declare_dram_parameter("bqk", [4 * HD, 1], f32, isOutput=False)
    bvb = nc.declare_dram_parameter("bvb", [128, 2 * HD], f16, isOutput=False)
    cosT = nc.declare_dram_parameter("cosT", [R, L], f16, isOutput=False)
    sinT = nc.declare_dram_parameter("sinT", [R, L], f16, isOutput=False)
    masks = nc.declare_dram_parameter("masks", [128, 128], f16, isOutput=False)
    wd = nc.declare_dram_parameter("wd", [D, D], f16, isOutput=False)
    bdb = nc.declare_dram_parameter("bdb", [128, D], f32, isOutput=False)
    y = nc.declare_dram_parameter("y", [M // W, D], f32, isOutput=True)

    xT_r = xT.ap().rearrange("(t p) n -> p t n", p=128)   # [128, 16, M]
    wd_r = wd.ap().rearrange("(t p) n -> p t n", p=128)

    with tile.TileContext(nc) as tc:
        with (
            tc.tile_pool(name="const", bufs=1) as cpool,
            tc.tile_pool(name="dram", bufs=1, space="DRAM") as dpool,
            tc.tile_pool(name="ps", bufs=1, space="PSUM") as pp,
            tc.tile_pool(name="qkv", bufs=1) as qkvpool,
            tc.tile_pool(name="p3", bufs=1) as p3pool,      # low in stack: prefetchable
            tc.tile_pool(name="p3s", bufs=2) as p3s,
            tc.tile_pool(name="att", bufs=3) as apool,
            tc.tile_pool(name="p1", bufs=3) as p1pool,
        ):
            a2a_ins = [[dpool.tile([W, HD, 256], f16, name=f"a2ain{b}h{h}")
                        for h in range(2)] for b in range(B)]
            a2a_outs = [[dpool.tile([W, HD, 256], f16, name=f"a2aout{b}h{h}")
                         for h in range(2)] for b in range(B)]

            # W load split so early k-tiles land first
            wq_r = wqkv.ap().rearrange("(t p) m -> p t m", p=128)
            w_sbs = [cpool.tile([128, 4, 6 * HD], f16, name=f"w{wq}")
                     for wq in range(4)]
            nc.sync.dma_start(out=w_sbs[0][:], in_=wq_r[:, 0:4, :])
            bqk_sb = cpool.tile([128, 4], f32)
            bvb_sb = cpool.tile([128, 2 * HD], f16)
            ones2t = cpool.tile([128, 2, 16], f8)
            nc.vector.memset(ones2t[:], 1.0)
            ones2 = ones2t[:, :, 0:1]     # [128, 2, 1], 16B j-stride for DR
            expb = cpool.tile([128, 1], f32)
            nc.vector.memset(expb[:], EXPB)
            cos_sb = cpool.tile([R, L], f16)
            sin_sb = cpool.tile([R, L], f16)
            mask_sb = cpool.tile([128, 128], f16)
            consts_loaded = False

            qk_sbs, v_sbs = [], []
            for b in range(B):
                qk_sbs.append(qkvpool.tile([128, 4, L], f16, name=f"qk{b}"))
                v_sbs.append(qkvpool.tile([128, 16, 2 * HD], f16, name=f"v{b}"))

            # ---- phase 1 (both batches) + fused RoPE ----
            for b in range(B):
                qk_sb, v_sb = qk_sbs[b], v_sbs[b]
                for nch in range(L // 512):
                    n0 = b * L + nch * 512
                    ch = slice(nch * 512, (nch + 1) * 512)
                    xt_tiles = []
                    for half in range(2):
                        xt = p1pool.tile([128, 8, 512], f16, tag="xt", bufs=3)
                        nc.sync.dma_start(
                            out=xt[:],
                            in_=xT_r[:, half * 8:(half + 1) * 8, n0:n0 + 512],
                        )
                        xt_tiles.append(xt)
                    if not consts_loaded:
                        nc.sync.dma_start(
                            out=bqk_sb[:],
                            in_=bqk.ap().rearrange("(t p) o -> p (t o)", p=128))
                        nc.sync.dma_start(out=bvb_sb[:], in_=bvb.ap())
                        for wq in range(1, 4):
                            nc.sync.dma_start(
                                out=w_sbs[wq][:],
                                in_=wq_r[:, 4 * wq:4 * (wq + 1), :],
                            )
                        nc.sync.dma_start(out=cos_sb[:], in_=cosT.ap())
                        nc.sync.dma_start(out=sin_sb[:], in_=sinT.ap())
                        nc.sync.dma_start(out=mask_sb[:], in_=masks.ap())
                        consts_loaded = True
                    for mp in range(2):
                        pss2 = pp.tile([128, 1024], f32, tag="work2", bufs=2,
                                       name=f"qkps{b}_{nch}_{mp}")
                        for kt in range(16):
                            xt = xt_tiles[kt // 8]
                            for i in range(2):
                                m = 2 * mp + i
                                nc.tensor.matmul(
                                    pss2[:, i * 512:(i + 1) * 512],
                                    lhsT=w_sbs[kt // 4][:, kt % 4, m * 128:(m + 1) * 128],
                                    rhs=xt[:, kt % 8, :],
                                    start=(kt == 0), stop=(kt == 15),
                                )
                        for i in range(2):
                            m = 2 * mp + i
                            nc.vector.tensor_scalar_add(
                                qk_sb[:, m, ch],
                                pss2[:, i * 512:(i + 1) * 512],
                                bqk_sb[:, m:m + 1],
                            )
                    for m in range(4):
                        # fused RoPE on rows 0:R of this chunk (fp16: DVE 2x)
                        cs = cos_sb[:, nch * 512:nch * 512 + 512]
                        sn = sin_sb[:, nch * 512:nch * 512 + 512]
                        ta = p1pool.tile([R, 512], f16, tag="ta", bufs=1)
                        rot = p1pool.tile([R, 512], f16, tag="rot", bufs=1)
                        tb = p1pool.tile([R, 512], f16, tag="tb", bufs=1)
                        nc.sync.dma_start(out=rot[0:16, :], in_=qk_sb[16:32, m, ch])
                        nc.sync.dma_start(out=rot[16:32, :], in_=qk_sb[0:16, m, ch])
                        nc.vector.tensor_tensor(
                            ta[:], qk_sb[0:R, m, ch], cs, op=OP.mult
                        )
                        nc.vector.tensor_tensor(tb[:], rot[:], sn, op=OP.mult)
                        nc.vector.tensor_tensor(
                            qk_sb[0:R, m, ch], ta[:], tb[:], op=OP.add
                        )
                    for rr2 in range(2):
                        vpss = [
                            pp.tile([128, 2 * HD], f32, tag="acc", bufs=3,
                                    name=f"vps{b}_{nch}_{2 * rr2 + i}")
                            for i in range(2)
                        ]
                        for kt in range(16):
                            xt = xt_tiles[kt // 8]
                            for i in range(2):
                                rr = 2 * rr2 + i
                                nc.tensor.matmul(
                                    vpss[i][:],
                                    lhsT=xt[:, kt % 8, rr * 128:(rr + 1) * 128],
                                    rhs=w_sbs[kt // 4][:, kt % 4, 4 * HD:6 * HD],
                                    start=(kt == 0), stop=(kt == 15),
                                )
                        for i in range(2):
                            rr = 2 * rr2 + i
                            nc.vector.tensor_tensor(
                                v_sb[:, nch * 4 + rr, :], vpss[i][:],
                                bvb_sb[:], op=OP.add,
                            )

            # phase-3 prefetches (pool low in the stack)
            bd_sb = p3pool.tile([128, D], f32)
            nc.gpsimd.dma_start(out=bd_sb[:], in_=bdb.ap())
            o_sbE = [p3pool.tile([128, 8, 256], f16, name=f"osbE{b}")
                     for b in range(B)]
            o_sbO = [p3pool.tile([128, 8, 256], f16, name=f"osbO{b}")
                     for b in range(B)]

            def wd_tile(n4):
                wt = p3s.tile([128, 16, 512], f16, tag="wdt", bufs=3,
                              name=f"wd{n4}")
                nc.sync.dma_start(out=wt[:, 0:8, :],
                                  in_=wd_r[:, 0:8, n4 * 512:(n4 + 1) * 512])
                nc.sync.dma_start(out=wt[:, 8:16, :],
                                  in_=wd_r[:, 8:16, n4 * 512:(n4 + 1) * 512])
                return wt

            def p3_mms(yp, bh, i, g, wt):
                osb = o_sbE[bh] if g == 0 else o_sbO[bh]
                for t8 in range(8):
                    nc.tensor.matmul(
                        yp[:],
                        lhsT=osb[:, t8, i * 128:(i + 1) * 128],
                        rhs=wt[:, 2 * t8 + g, :],
                        start=(t8 == 0), stop=(t8 == 7),
                    )

            def p3_out(m, n4, yp, extra):
                yt = p3s.tile([128, 512], f32, tag="yt", bufs=3)
                nc.vector.tensor_tensor(
                    yt[:], yp[:], bd_sb[:, n4 * 512:(n4 + 1) * 512], op=OP.add
                )
                if extra is not None:
                    nc.vector.tensor_tensor(yt[:], yt[:], extra[:], op=OP.add)
                nc.scalar.dma_start(
                    out=y[m * 128:(m + 1) * 128, n4 * 512:(n4 + 1) * 512],
                    in_=yt[:],
                )

            def p3_unit(bh, i, n4, wt):
                yp = pp.tile([128, 512], f32, tag="acc", bufs=3,
                             name=f"yps{bh}_{i}_{n4}")
                for g in range(2):
                    if g == 0:
                        osb = o_sbE[bh]
                    else:
                        osb = o_sbO[bh]
                    for t8 in range(8):
                        nc.tensor.matmul(
                            yp[:],
                            lhsT=osb[:, t8, i * 128:(i + 1) * 128],
                            rhs=wt[:, 2 * t8 + g, :],
                            start=(g == 0 and t8 == 0), stop=(g == 1 and t8 == 7),
                        )
                p3_out(2 * bh + i, n4, yp, None)

            # phase-3 b0 units: (n4-major so each wd tile serves 2 units)
            p3b0_queue = [(0, i, n4) for n4 in range(4) for i in range(2)]
            p3_wd = {}
            p3b1_pre = {}        # (i, n4) -> SBUF staged even-half result

            def emit_p3b0(k):
                for _ in range(k):
                    if not p3b0_queue:
                        return
                    bh, i, n4 = p3b0_queue.pop(0)
                    if n4 not in p3_wd:
                        p3_wd[n4] = wd_tile(n4)
                    p3_unit(bh, i, n4, p3_wd[n4])

            def emit_p3b1_even():
                # even-kt halves for n4 0..1 during late b1-h1 attention;
                # staged to SBUF so PSUM frees before the tail
                for n4 in range(2):
                    wt = wd_tile(n4)
                    p3_wd[(1, n4)] = wt
                    for i in range(2):
                        yp = pp.tile([128, 512], f32, tag="acc", bufs=3,
                                     name=f"ypre{i}_{n4}")
                        p3_mms(yp, 1, i, 0, wt)
                        ye = p3s.tile([128, 512], f32, tag=f"ye{i}_{n4}",
                                      bufs=1, name=f"ye{i}_{n4}")
                        nc.vector.tensor_copy(ye[:], yp[:])
                        p3b1_pre[(i, n4)] = ye

            # ---- phase 2: attention, per batch ----
            for b in range(B):
                qk_sb, v_sb = qk_sbs[b], v_sbs[b]
                it = 0
                for h in range(HPC):
                    for qc in reversed(range(L // 512)):
                        nk = 4 * qc + 4
                        npairs = nk // 2
                        outp = pp.tile([128, 512], f32, tag="acc", bufs=3,
                                       name=f"outp{b}_{h}_{qc}")
                        sump = pp.tile([1, 512], f32, tag="sump", bufs=1,
                                       name=f"sump{b}_{h}_{qc}")

                        def emit_scores(pi):
                            sp2 = pp.tile([128, 1024], f32, tag="work2", bufs=2,
                                          name=f"sp{b}_{h}_{qc}_{pi}")
                            et2 = apool.tile([128, 2, 512], f8, tag="et", bufs=4)
                            tl = []
                            for j in range(2):
                                ki = 2 * pi + j
                                c0 = max(0, ki - qc * 4) * 128
                                tl.append((j, ki, c0))
                                qs = slice(qc * 512 + c0, (qc + 1) * 512)
                                nc.tensor.matmul(
                                    sp2[:, j * 512 + c0:(j + 1) * 512],
                                    lhsT=qk_sb[:, 2 * h + 1,
                                               ki * 128:(ki + 1) * 128],
                                    rhs=qk_sb[:, 2 * h, qs],
                                    start=True, stop=True,
                                )
                            if tl[1][2] == 0:
                                nc.scalar.activation(
                                    et2[:],
                                    sp2[:].rearrange("p (j n) -> p j n", j=2),
                                    AFT.Exp, scale=SCALE, bias=expb[:],
                                )
                            else:
                                for j, ki, c0 in tl:
                                    nc.scalar.activation(
                                        et2[:, j, c0:512],
                                        sp2[:, j * 512 + c0:(j + 1) * 512],
                                        AFT.Exp, scale=SCALE, bias=expb[:],
                                    )
                                c0a, c0b = tl[0][2], tl[1][2]
                                if c0b > c0a:
                                    nc.vector.memset(et2[:, 1, c0a:c0b], 0.0)
                            for j, ki, c0 in tl:
                                if ki >= qc * 4:
                                    nc.vector.tensor_tensor(
                                        et2[:, j, c0:c0 + 128],
                                        et2[:, j, c0:c0 + 128],
                                        mask_sb[:], op=OP.mult,
                                    )
                            return pi, et2, tl

                        def emit_av(pi, et2, tl):
                            for j, ki, c0 in tl:
                                nc.tensor.matmul(
                                    outp[:, c0:512],
                                    lhsT=v_sb[:, ki, h * 128:(h + 1) * 128],
                                    rhs=et2[:, j, c0:512],
                                    start=(ki == 0), stop=(ki == nk - 1),
                                )
                            c0a = tl[0][2]
                            nc.tensor.matmul(
                                sump[:, c0a:512], lhsT=ones2[:],
                                rhs=et2[:, :, c0a:512],
                                start=(pi == 0), stop=(pi == npairs - 1),
                                perf_mode=DR,
                            )

                        pend = None
                        for pi in range(npairs):
                            cur = emit_scores(pi)
                            if pend is not None:
                                emit_av(*pend)
                            pend = cur
                        emit_av(*pend)

                        rec = apool.tile([1, 512], f32, tag="rec", bufs=2)
                        nc.vector.reciprocal_approx_fast(rec[:], sump[:])
                        bcs = apool.tile([128, 512], f32, tag="bcs", bufs=1)
                        nc.gpsimd.partition_broadcast(bcs[:], rec[:])
                        ot = apool.tile([128, 512], f16, tag="ot", bufs=2)
                        nc.vector.tensor_tensor(ot[:], outp[:], bcs[:],
                                                op=OP.mult)
                        for half in range(2):
                            nc.sync.dma_start(
                                out=a2a_ins[b][h][2 * qc + half, :, :],
                                in_=ot[:, half * 256:(half + 1) * 256],
                            )
                        it += 1
                        if b == 1 and it >= 3:
                            emit_p3b0(2)
                        if b == 1 and it == 8:
                            emit_p3b1_even()
                    # A2A for this head's half right after its attention
                    nc.gpsimd.collective_compute(
                        "AllToAll", mybir.AluOpType.bypass,
                        replica_groups=[CORES],
                        ins=[a2a_ins[b][h][:]], outs=[a2a_outs[b][h][:]],
                    )
                    dstEO = o_sbE[b] if h == 0 else o_sbO[b]
                    nc.sync.dma_start(
                        out=dstEO[:],
                        in_=a2a_outs[b][h][:].rearrange("j p n -> p j n"),
                    )

            # ---- phase 3 tail: leftover b0 units + all b1 units ----
            emit_p3b0(len(p3b0_queue))
            for n4 in range(4):
                if (1, n4) in p3_wd:
                    wt = p3_wd[(1, n4)]
                    for i in range(2):
                        yp = pp.tile([128, 512], f32, tag="acc", bufs=3,
                                     name=f"ypo{i}_{n4}")
                        p3_mms(yp, 1, i, 1, wt)
                        p3_out(2 + i, n4, yp, p3b1_pre[(i, n4)])
                else:
                    wt = wd_tile(n4)
                    for i in range(2):
                        p3_unit(1, i, n4, wt)
    nc.finalize()
    return nc


def _host_prep(x_BLD, cos, sin, Wqkv, bqkv, Wd, bd):
    x = np.asarray(x_BLD, np.float32).reshape(M, D)
    xT = np.ascontiguousarray(x.T.astype(np.float16))
    c2 = np.asarray(cos, np.float32).reshape(L, R).T
    s2 = np.asarray(sin, np.float32).reshape(L, R).T
    cosT = np.ascontiguousarray(c2)
    sinT_pm = np.ascontiguousarray(
        np.concatenate([-s2[:16], s2[16:]], axis=0)
    )
    kk = np.arange(128, dtype=np.int64)[:, None]
    qq = np.arange(128, dtype=np.int64)[None, :]
    masks = (qq >= kk).astype(np.float16)
    bdb = np.ascontiguousarray(
        np.broadcast_to(np.asarray(bd, np.float32), (128, D))
    )
    Wqkv = np.asarray(Wqkv, np.float32)
    bqkv = np.asarray(bqkv, np.float32)
    in_maps = []
    for c in range(W):
        base = c * HPC * 3 * HD
        qk_idx = np.concatenate(
            [np.arange(base + h * 3 * HD, base + h * 3 * HD + 2 * HD)
             for h in range(HPC)]
        )
        v_idx = np.concatenate(
            [np.arange(base + h * 3 * HD + 2 * HD, base + (h + 1) * 3 * HD)
             for h in range(HPC)]
        )
        in_maps.append({
            "xT": xT,
            "wqkv": np.ascontiguousarray(
                Wqkv[:, np.concatenate([qk_idx, v_idx])].astype(np.float16)
            ),
            "bqk": np.ascontiguousarray(bqkv[qk_idx].reshape(4 * HD, 1)),
            "bvb": np.ascontiguousarray(np.broadcast_to(
                bqkv[v_idx].reshape(1, 2 * HD).astype(np.float16),
                (128, 2 * HD),
            )),
            "cosT": cosT.astype(np.float16),
            "sinT": sinT_pm.astype(np.float16),
            "masks": masks,
            "wd": np.asarray(Wd, np.float32).astype(np.float16),
            "bdb": bdb,
        })
    return in_maps


def _get_nc():
    global _NC
    if _NC is None:
        _NC = _build_nc()
    return _NC


def _run(inputs, trace=False, tmpdir=None):
    from concourse.bass_utils import run_bass_kernel_spmd

    in_maps = _host_prep(**inputs)
    nc = _get_nc()
    res = run_bass_kernel_spmd(nc, in_maps, CORES, trace=trace, tmpdir=tmpdir)
    out = np.empty((M, D), np.float32)
    for c in CORES:
        yc = res.results[c]["y"]          # [512, D]: rows b0 then b1
        out[c * 256:(c + 1) * 256] = yc[:256]
        out[L + c * 256:L + (c + 1) * 256] = yc[256:]
    return out.reshape(B, L, D), res


def kernel(**inputs) -> np.ndarray:
    out, _ = _run(inputs)
    return out


# revision 24
# speedup vs baseline: 1.0462x; 1.0462x over previous
"""Distributed Trainium2 kernel for nn_DecoderAttentionRotary.

Strategy (8 NeuronCores, tensor-parallel over heads, fp16 matmuls + fp8 et):
  - host: transpose x -> xT [D, B*L] fp16; per-core Wqkv column slice
    reordered to [q0,k0,q1,k1,v0|v1] fp16; cos/sin fp16; causal masks fp16.
  - device, per core (2 heads):
      phase 1: qkT = (Wqk^T @ xT) + b (fp16, double-wide [128,1024] psums),
               v = x @ Wv in [l, hd] layout; RoPE fused (DVE fp16 2x mode).
      phase 2: causal attention in scores^T layout; one exp per k-tile PAIR
               (ACT -> fp8e4 et, bias=-2 keeps fp8 range, cancels in norm);
               AV per tile (fp16 v x fp8 et), rowsums via one fp8-DoubleRow
               ones-matmul per pair; normalize = approx-reciprocal + gpsimd
               partition_broadcast; scores of pair i+1 emitted before AV i.
      A2A b0 after batch 0 (overlaps b1 attention); b1's A2A split per head
      (h0 half overlaps h1 attention).  phase-3 b0 units interleaved into
      the b1 attention emission (PE fills ACT-bound gaps).
      phase 3: y = o^T @ Wd + bd per (i, n4) unit; even-kt matmuls first so
      b1 units start after the h0 half-A2A; Wd streamed through 2 slots.
  - host: scatter the per-core 256-row halves into the full output.
"""
import sys

for _p in ("/opt/pypackages", "/opt/trn_rl_repo"):
    if _p not in sys.path:
        sys.path.insert(0, _p)

import numpy as np

B, L, D, H = 2, 2048, 2048, 16
HD, R = 128, 32
SCALE = float(HD) ** -0.5
EXPB = -2.0               # exp bias: keeps fp8e4 et in range; cancels in norm
W = 8
HPC = H // W              # heads per core
M = B * L                 # flattened rows
CORES = list(range(W))

_NC = None


def _build_nc():
    import concourse.mybir as mybir
    import concourse.tile as tile
    from concourse import bacc

    f32 = mybir.dt.float32
    f16 = mybir.dt.float16
    f8 = mybir.dt.float8e4
    DR = mybir.MatmulPerfMode.DoubleRow
    AFT = mybir.ActivationFunctionType
    OP = mybir.AluOpType

    nc = bacc.Bacc(None, target_bir_lowering=False, num_devices=W)
    xT = nc.declare_dram_parameter("xT", [D, M], f16, isOutput=False)
    wqkv = nc.declare_dram_parameter("wqkv", [D, 6 * HD], f16, isOutput=False)
    bqk = nc.declare_dram_parameter("bqk", [4 * HD, 1], f32, isOutput=False)
    bvb = nc.declare_dram_parameter("bvb", [128, 2 * HD], f16, isOutput=False)
    cosT = nc.declare_dram_parameter("cosT", [R, L], f16, isOutput=False)
    sinT = nc.declare_dram_parameter("sinT", [R, L], f16, isOutput=False)
    masks = nc.declare_dram_parameter("masks", [128, 128], f16, isOutput=False)
    wd = nc.declare_dram_parameter("wd", [D, D], f16, isOutput=False)
    bdb = nc.declare_dram_parameter("bdb", [128, D], f32, isOutput=False)
    y = nc.declare_dram_parameter("y", [M // W, D], f32, isOutput=True)

    xT_r = xT.ap().rearrange("(t p) n -> p t n", p=128)   # [128, 16, M]
    wd_r = wd.ap().rearrange("(t p) n -> p t n", p=128)

    with tile.TileContext(nc) as tc:
        with (
            tc.tile_pool(name="const", bufs=1) as cpool,
            tc.tile_pool(name="dram", bufs=1, space="DRAM") as dpool,
            tc.tile_pool(name="ps", bufs=1, space="PSUM") as pp,
            tc.tile_pool(name="qkv", bufs=1) as qkvpool,
            tc.tile_pool(name="p3", bufs=1) as p3pool,      # low in stack: prefetchable
            tc.tile_pool(name="p3s", bufs=2) as p3s,
            tc.tile_pool(name="att", bufs=3) as apool,
            tc.tile_pool(name="p1", bufs=3) as p1pool,
        ):
            a2a_ins = [[dpool.tile([W, HD, 256], f16, name=f"a2ain{b}h{h}")
                        for h in range(2)] for b in range(B)]
            a2a_outs = [[dpool.tile([W, HD, 256], f16, name=f"a2aout{b}h{h}")
                         for h in range(2)] for b in range(B)]

            # W load split so early k-tiles land first
            wq_r = wqkv.ap().rearrange("(t p) m -> p t m", p=128)
            w_sbs = [cpool.tile([128, 4, 6 * HD], f16, name=f"w{wq}")
                     for wq in range(4)]
            nc.sync.dma_start(out=w_sbs[0][:], in_=wq_r[:, 0:4, :])
            bqk_sb = cpool.tile([128, 4], f32)
            bvb_sb = cpool.tile([128, 2 * HD], f16)
            ones2t = cpool.tile([128, 2, 16], f8)
            nc.vector.memset(ones2t[:], 1.0)
            ones2 = ones2t[:, :, 0:1]     # [128, 2, 1], 16B j-stride for DR
            expb = cpool.tile([128, 1], f32)
            nc.vector.memset(expb[:], EXPB)
            cos_sb = cpool.tile([R, L], f16)
            sin_sb = cpool.tile([R, L], f16)
            mask_sb = cpool.tile([128, 128], f16)
            consts_loaded = False

            qk_sbs, v_sbs = [], []
            for b in range(B):
                qk_sbs.append(qkvpool.tile([128, 4, L], f16, name=f"qk{b}"))
                v_sbs.append(qkvpool.tile([128, 16, 2 * HD], f16, name=f"v{b}"))

            # ---- phase 1 (both batches) + fused RoPE ----
            for b in range(B):
                qk_sb, v_sb = qk_sbs[b], v_sbs[b]
                for nch in range(L // 512):
                    n0 = b * L + nch * 512
                    ch = slice(nch * 512, (nch + 1) * 512)
                    xt_tiles = []
                    for half in range(2):
                        xt = p1pool.tile([128, 8, 512], f16, tag="xt", bufs=3)
                        nc.sync.dma_start(
                            out=xt[:],
                            in_=xT_r[:, half * 8:(half + 1) * 8, n0:n0 + 512],
                        )
                        xt_tiles.append(xt)
                    if not consts_loaded:
                        nc.sync.dma_start(
                            out=bqk_sb[:],
                            in_=bqk.ap().rearrange("(t p) o -> p (t o)", p=128))
                        nc.sync.dma_start(out=bvb_sb[:], in_=bvb.ap())
                        for wq in range(1, 4):
                            nc.sync.dma_start(
                                out=w_sbs[wq][:],
                                in_=wq_r[:, 4 * wq:4 * (wq + 1), :],
                            )
                        nc.sync.dma_start(out=cos_sb[:], in_=cosT.ap())
                        nc.sync.dma_start(out=sin_sb[:], in_=sinT.ap())
                        nc.sync.dma_start(out=mask_sb[:], in_=masks.ap())
                        consts_loaded = True
                    for mp in range(2):
                        pss2 = pp.tile([128, 1024], f32, tag="work2", bufs=2,
                                       name=f"qkps{b}_{nch}_{mp}")
                        for kt in range(16):
                            xt = xt_tiles[kt // 8]
                            for i in range(2):
                                m = 2 * mp + i
                                nc.tensor.matmul(
                                    pss2[:, i * 512:(i + 1) * 512],
                                    lhsT=w_sbs[kt // 4][:, kt % 4, m * 128:(m + 1) * 128],
                                    rhs=xt[:, kt % 8, :],
                                    start=(kt == 0), stop=(kt == 15),
                                )
                        for i in range(2):
                            m = 2 * mp + i
                            nc.vector.tensor_scalar_add(
                                qk_sb[:, m, ch],
                                pss2[:, i * 512:(i + 1) * 512],
                                bqk_sb[:, m:m + 1],
                            )
                    for m in range(4):
                        # fused RoPE on rows 0:R of this chunk (fp16: DVE 2x)
                        cs = cos_sb[:, nch * 512:nch * 512 + 512]
                        sn = sin_sb[:, nch * 512:nch * 512 + 512]
                        ta = p1pool.tile([R, 512], f16, tag="ta", bufs=1)
                        rot = p1pool.tile([R, 512], f16, tag="rot", bufs=1)
                        tb = p1pool.tile([R, 512], f16, tag="tb", bufs=1)
                        nc.sync.dma_start(out=rot[0:16, :], in_=qk_sb[16:32, m, ch])
                        nc.sync.dma_start(out=rot[16:32, :], in_=qk_sb[0:16, m, ch])
                        nc.vector.tensor_tensor(
                            ta[:], qk_sb[0:R, m, ch], cs, op=OP.mult
                        )
                        nc.vector.tensor_tensor(tb[:], rot[:], sn, op=OP.mult)
                        nc.vector.tensor_tensor(
                            qk_sb[0:R, m, ch], ta[:], tb[:], op=OP.add
                        )
                    for rr2 in range(2):
                        vpss = [
                            pp.tile([128, 2 * HD], f32, tag="acc", bufs=3,
                                    name=f"vps{b}_{nch}_{2 * rr2 + i}")
                            for i in range(2)
                        ]
                        for kt in range(16):
                            xt = xt_tiles[kt // 8]
                            for i in range(2):
                                rr = 2 * rr2 + i
                                nc.tensor.matmul(
                                    vpss[i][:],
                                    lhsT=xt[:, kt % 8, rr * 128:(rr + 1) * 128],
                                    rhs=w_sbs[kt // 4][:, kt % 4, 4 * HD:6 * HD],
                                    start=(kt == 0), stop=(kt == 15),
                                )
                        for i in range(2):
                            rr = 2 * rr2 + i
                            nc.vector.tensor_tensor(
                                v_sb[:, nch * 4 + rr, :], vpss[i][:],
                                bvb_sb[:], op=OP.add,
                            )

            # phase-3 prefetches (pool low in the stack)
            bd_sb = p3pool.tile([128, D], f32)
            nc.gpsimd.dma_start(out=bd_sb[:], in_=bdb.ap())
            o_sbE = [p3pool.tile([128, 8, 256], f16, name=f"osbE{b}")
                     for b in range(B)]
            o_sbO = [p3pool.tile([128, 8, 256], f16, name=f"osbO{b}")
                     for b in range(B)]

            def wd_tile(n4):
                wt = p3s.tile([128, 16, 512], f16, tag="wdt", bufs=3,
                              name=f"wd{n4}")
                nc.sync.dma_start(out=wt[:, 0:8, :],
                                  in_=wd_r[:, 0:8, n4 * 512:(n4 + 1) * 512])
                nc.sync.dma_start(out=wt[:, 8:16, :],
                                  in_=wd_r[:, 8:16, n4 * 512:(n4 + 1) * 512])
                return wt

            def p3_mms(yp, bh, i, g, wt):
                osb = o_sbE[bh] if g == 0 else o_sbO[bh]
                for t8 in range(8):
                    nc.tensor.matmul(
                        yp[:],
                        lhsT=osb[:, t8, i * 128:(i + 1) * 128],
                        rhs=wt[:, 2 * t8 + g, :],
                        start=(t8 == 0), stop=(t8 == 7),
                    )

            def p3_out(m, n4, yp, extra):
                yt = p3s.tile([128, 512], f32, tag="yt", bufs=2)
                nc.vector.tensor_tensor(
                    yt[:], yp[:], bd_sb[:, n4 * 512:(n4 + 1) * 512], op=OP.add
                )
                if extra is not None:
                    nc.vector.tensor_tensor(yt[:], yt[:], extra[:], op=OP.add)
                nc.scalar.dma_start(
                    out=y[m * 128:(m + 1) * 128, n4 * 512:(n4 + 1) * 512],
                    in_=yt[:],
                )

            def p3_unit(bh, i, n4, wt):
                yp = pp.tile([128, 512], f32, tag="acc", bufs=3,
                             name=f"yps{bh}_{i}_{n4}")
                for g in range(2):
                    if g == 0:
                        osb = o_sbE[bh]
                    else:
                        osb = o_sbO[bh]
                    for t8 in range(8):
                        nc.tensor.matmul(
                            yp[:],
                            lhsT=osb[:, t8, i * 128:(i + 1) * 128],
                            rhs=wt[:, 2 * t8 + g, :],
                            start=(g == 0 and t8 == 0), stop=(g == 1 and t8 == 7),
                        )
                p3_out(2 * bh + i, n4, yp, None)

            # phase-3 b0 units: (n4-major so each wd tile serves 2 units)
            p3b0_queue = [(0, i, n4) for n4 in range(4) for i in range(2)]
            p3_wd = {}
            p3b1_pre = {}        # (i, n4) -> SBUF staged even-half result

            def emit_p3b0(k):
                for _ in range(k):
                    if not p3b0_queue:
                        return
                    bh, i, n4 = p3b0_queue.pop(0)
                    if n4 not in p3_wd:
                        p3_wd[n4] = wd_tile(n4)
                    p3_unit(bh, i, n4, p3_wd[n4])

            def emit_p3b1_even():
                # even-kt halves for n4 0..1 during late b1-h1 attention;
                # staged to SBUF so PSUM frees before the tail
                for n4 in range(2):
                    wt = wd_tile(n4)
                    p3_wd[(1, n4)] = wt
                    for i in range(2):
                        yp = pp.tile([128, 512], f32, tag="acc", bufs=3,
                                     name=f"ypre{i}_{n4}")
                        p3_mms(yp, 1, i, 0, wt)
                        ye = p3s.tile([128, 512], f32, tag=f"ye{i}_{n4}",
                                      bufs=1, name=f"ye{i}_{n4}")
                        nc.vector.tensor_copy(ye[:], yp[:])
                        p3b1_pre[(i, n4)] = ye

            # ---- phase 2: attention, per batch ----
            for b in range(B):
                qk_sb, v_sb = qk_sbs[b], v_sbs[b]
                it = 0
                for h in range(HPC):
                    for qc in reversed(range(L // 512)):
                        nk = 4 * qc + 4
                        npairs = nk // 2
                        outp = pp.tile([128, 512], f32, tag="acc", bufs=3,
                                       name=f"outp{b}_{h}_{qc}")
                        sump = pp.tile([1, 512], f32, tag="sump", bufs=1,
                                       name=f"sump{b}_{h}_{qc}")

                        def emit_scores(pi):
                            sp2 = pp.tile([128, 1024], f32, tag="work2", bufs=2,
                                          name=f"sp{b}_{h}_{qc}_{pi}")
                            et2 = apool.tile([128, 2, 512], f8, tag="et", bufs=3)
                            tl = []
                            for j in range(2):
                                ki = 2 * pi + j
                                c0 = max(0, ki - qc * 4) * 128
                                tl.append((j, ki, c0))
                                qs = slice(qc * 512 + c0, (qc + 1) * 512)
                                nc.tensor.matmul(
                                    sp2[:, j * 512 + c0:(j + 1) * 512],
                                    lhsT=qk_sb[:, 2 * h + 1,
                                               ki * 128:(ki + 1) * 128],
                                    rhs=qk_sb[:, 2 * h, qs],
                                    start=True, stop=True,
                                )
                            if tl[1][2] == 0:
                                nc.scalar.activation(
                                    et2[:],
                                    sp2[:].rearrange("p (j n) -> p j n", j=2),
                                    AFT.Exp, scale=SCALE, bias=expb[:],
                                )
                            else:
                                for j, ki, c0 in tl:
                                    nc.scalar.activation(
                                        et2[:, j, c0:512],
                                        sp2[:, j * 512 + c0:(j + 1) * 512],
                                        AFT.Exp, scale=SCALE, bias=expb[:],
                                    )
                                c0a, c0b = tl[0][2], tl[1][2]
                                if c0b > c0a:
                                    nc.vector.memset(et2[:, 1, c0a:c0b], 0.0)
                            for j, ki, c0 in tl:
                                if ki >= qc * 4:
                                    nc.vector.tensor_tensor(
                                        et2[:, j, c0:c0 + 128],
                                        et2[:, j, c0:c0 + 128],
                                        mask_sb[:], op=OP.mult,
                                    )
                            return pi, et2, tl

                        def emit_av(pi, et2, tl):
                            for j, ki, c0 in tl:
                                nc.tensor.matmul(
                                    outp[:, c0:512],
                                    lhsT=v_sb[:, ki, h * 128:(h + 1) * 128],
                                    rhs=et2[:, j, c0:512],
                                    start=(ki == 0), stop=(ki == nk - 1),
                                )
                            c0a = tl[0][2]
                            nc.tensor.matmul(
                                sump[:, c0a:512], lhsT=ones2[:],
                                rhs=et2[:, :, c0a:512],
                                start=(pi == 0), stop=(pi == npairs - 1),
                                perf_mode=DR,
                            )

                        pend = None
                        for pi in range(npairs):
                            cur = emit_scores(pi)
                            if pend is not None:
                                emit_av(*pend)
                            pend = cur
                        emit_av(*pend)

                        rec = apool.tile([1, 512], f32, tag="rec", bufs=2)
                        nc.vector.reciprocal_approx_fast(rec[:], sump[:])
                        bcs = apool.tile([128, 512], f32, tag="bcs", bufs=1)
                        nc.gpsimd.partition_broadcast(bcs[:], rec[:])
                        ot = apool.tile([128, 512], f16, tag="ot", bufs=2)
                        nc.vector.tensor_tensor(ot[:], outp[:], bcs[:],
                                                op=OP.mult)
                        for half in range(2):
                            nc.sync.dma_start(
                                out=a2a_ins[b][h][2 * qc + half, :, :],
                                in_=ot[:, half * 256:(half + 1) * 256],
                            )
                        it += 1
                        if b == 1 and it >= 3:
                            emit_p3b0(2)
                        if b == 1 and it == 8:
                            emit_p3b1_even()
                    # A2A for this head's half right after its attention
                    nc.gpsimd.collective_compute(
                        "AllToAll", mybir.AluOpType.bypass,
                        replica_groups=[CORES],
                        ins=[a2a_ins[b][h][:]], outs=[a2a_outs[b][h][:]],
                    )
                    dstEO = o_sbE[b] if h == 0 else o_sbO[b]
                    nc.sync.dma_start(
                        out=dstEO[:],
                        in_=a2a_outs[b][h][:].rearrange("j p n -> p j n"),
                    )

            # ---- phase 3 tail: leftover b0 units + all b1 units ----
            emit_p3b0(len(p3b0_queue))
            for n4 in range(4):
                if (1, n4) in p3_wd:
                    wt = p3_wd[(1, n4)]
                    for i in range(2):
                        yp = pp.tile([128, 512], f32, tag="acc", bufs=3,
                                     name=f"ypo{i}_{n4}")
                        p3_mms(yp, 1, i, 1, wt)
                        p3_out(2 + i, n4, yp, p3b1_pre[(i, n4)])
                else:
                    wt = wd_tile(n4)
                    for i in range(2):
                        p3_unit(1, i, n4, wt)
    nc.finalize()
    return nc


def _host_prep(x_BLD, cos, sin, Wqkv, bqkv, Wd, bd):
    x = np.asarray(x_BLD, np.float32).reshape(M, D)
    xT = np.ascontiguousarray(x.T.astype(np.float16))
    c2 = np.asarray(cos, np.float32).reshape(L, R).T
    s2 = np.asarray(sin, np.float32).reshape(L, R).T
    cosT = np.ascontiguousarray(c2)
    sinT_pm = np.ascontiguousarray(
        np.concatenate([-s2[:16], s2[16:]], axis=0)
    )
    kk = np.arange(128, dtype=np.int64)[:, None]
    qq = np.arange(128, dtype=np.int64)[None, :]
    masks = (qq >= kk).astype(np.float16)
    bdb = np.ascontiguousarray(
        np.broadcast_to(np.asarray(bd, np.float32), (128, D))
    )
    Wqkv = np.asarray(Wqkv, np.float32)
    bqkv = np.asarray(bqkv, np.float32)
    in_maps = []
    for c in range(W):
        base = c * HPC * 3 * HD
        qk_idx = np.concatenate(
            [np.arange(base + h * 3 * HD, base + h * 3 * HD + 2 * HD)
             for h in range(HPC)]
        )
        v_idx = np.concatenate(
            [np.arange(base + h * 3 * HD + 2 * HD, base + (h + 1) * 3 * HD)
             for h in range(HPC)]
        )
        in_maps.append({
            "xT": xT,
            "wqkv": np.ascontiguousarray(
                Wqkv[:, np.concatenate([qk_idx, v_idx])].astype(np.float16)
            ),
            "bqk": np.ascontiguousarray(bqkv[qk_idx].reshape(4 * HD, 1)),
            "bvb": np.ascontiguousarray(np.broadcast_to(
                bqkv[v_idx].reshape(1, 2 * HD).astype(np.float16),
                (128, 2 * HD),
            )),
            "cosT": cosT.astype(np.float16),
            "sinT": sinT_pm.astype(np.float16),
            "masks": masks,
            "wd": np.asarray(Wd, np.float32).astype(np.float16),
            "bdb": bdb,
        })
    return in_maps


def _get_nc():
    global _NC
    if _NC is None:
        _NC = _build_nc()
    return _NC


def _run(inputs, trace=False, tmpdir=None):
    from concourse.bass_utils import run_bass_kernel_spmd

    in_maps = _host_prep(**inputs)
    nc = _get_nc()
    res = run_bass_kernel_spmd(nc, in_maps, CORES, trace=trace, tmpdir=tmpdir)
    out = np.empty((M, D), np.float32)
    for c in CORES:
        yc = res.results[c]["y"]          # [512, D]: rows b0 then b1
        out[c * 256:(c + 1) * 256] = yc[:256]
        out[L + c * 256:L + (c + 1) * 256] = yc[256:]
    return out.reshape(B, L, D), res


def kernel(**inputs) -> np.ndarray:
    out, _ = _run(inputs)
    return out
